# revision 1
# baseline (speedup 1.0000x reference)
"""DeformConv2d Bass kernel for trn2 (8 NeuronCores, batch-sharded).

Algorithm (per core, one image, fp16 compute):
  1. offset conv (PE): off[27, HW] = sum_k Woff_k @ x_shift_k + b, with taps
     paired on the contraction dim (x + a column-shifted copy of x stacked on
     partitions 64:127) -> 6 matmuls per psum tile instead of 9.
  2. Y_k = W_dcn[:,:,k] @ x for the 9 kernel points (PE, 2 k per matmul pair),
     PE-transposed to [h-partitions, (o, w)] tiles (ACT drains).
  3. bilinear interp as dense 3-tap tent product:
       out[o,h,w] = sum_k sum_{ry,rx} u_{k,ry,rx}[h,w] * Y_k[o, h+ki+ry, w+kj+rx]
     u = sigmoid(logit) * tent(dy-ry) * tent(dx-rx), exact for |dy|,|dx| < 1.
  4. per-pixel multiplies run on DVE and Pool (greedy-balanced); the term
     accumulation runs on PE as shifted-identity matmuls accumulating in f32
     PSUM (the vertical shift a = ki+ry is baked into the stationary), folded
     into the fp16 SBUF accumulator Q by Pool/DVE. Vertically-unshifted (a=0)
     terms skip PSUM and add straight into Q on DVE/Pool.
"""

import numpy as np

B, CIN, COUT, H, W, K, PAD = 8, 64, 64, 128, 128, 3, 1
KK = K * K
HW = H * W            # 16384
XP = 130              # padded x row stride / rows
XSZ = XP * XP         # padded x elements per partition
WY = W + 4            # padded w-stride in transposed Y: 132 (w in -2..129)
PAIRS = [(0, 1), (2, 3), (4, 5), (6, 7), (8,)]
NE = 8                # FMA w-eighths
EW = W // NE          # 16 w-cols per eighth

# offset-conv tap pairing: within each ki row, (kj=-1, kj=0) share a matmul
# via the column-shifted x copy; kj=+1 runs alone on partitions 0:63.
# entries: (list of k's, ki, column offset into padded x)
OFF_MMS = []
for _ki in (-1, 0, 1):
    OFF_MMS.append(([3 * (_ki + 1) + 2], _ki, 2))                     # kj=+1
for _ki in (-1, 0, 1):
    OFF_MMS.append(([3 * (_ki + 1) + 0, 3 * (_ki + 1) + 1], _ki, 0))  # kj=-1 & kj=0

# u-field storage: one tensor per (ry, rx) holding all 9 k-blocks
# [h, (k, W)]; a second, per-k-band row-shifted copy serves the a != 0 terms.
RYRX = [(ry, rx) for ry in (-1, 0, 1) for rx in (-1, 0, 1)]
PGROUPS = [[0], [1], [2], [3, 4]]  # FMA groups; last two pairs fused

# ---- static engine plan -----------------------------------------------------
# Block-greedy balance of the elementwise work between DVE and Pool: whole
# eighth-blocks of PE-feeding mults and whole TT add-chains go to one engine,
# so each chain pipelines on a single in-order queue. Also decide which a=0
# terms bypass PSUM (direct tensor-tensor adds into Q).
def _pair_terms(pi):
    terms = []
    for k in PAIRS[pi]:
        ki, kj = k // 3 - 1, k % 3 - 1
        for ry in (-1, 0, 1):
            for rx in (-1, 0, 1):
                terms.append((k, ry, rx, ki + ry, kj + rx))
    return terms


def _plan():
    a0_by_pair = {pi: [t for t in _pair_terms(pi) if t[3] == 0]
                  for pi in range(len(PAIRS))}
    a0_rr = []
    for j in range(max(len(v) for v in a0_by_pair.values())):
        for pi in range(len(PAIRS)):
            if j < len(a0_by_pair[pi]):
                a0_rr.append((pi, a0_by_pair[pi][j]))
    n_pe_extra = 20  # a=0 terms routed through PE psum for balance
    pe_a0 = set()
    for pi, t in a0_rr[:n_pe_extra]:
        pe_a0.add((pi, t[0], t[1], t[2]))

    C_DVE = {2048: 1126.0, 1024: 593.0, 512: 593.0}   # fold(512) is 1x: 593
    C_POOL = {2048: 1706.0, 1024: 853.0, 512: 427.0}
    busy = {"v": 65000.0, "g": 90000.0}
    assign = {}

    def pick(key, free, nops):
        dv = busy["v"] + C_DVE[free] * nops
        pg = busy["g"] + C_POOL[free] * nops
        eng = "v" if dv <= pg else "g"
        busy[eng] = dv if eng == "v" else pg
        assign[key] = eng

    for gi, grp in enumerate(PGROUPS):
        terms = [(p, t) for p in grp for t in _pair_terms(p)]
        n_pe = sum(1 for p, t in terms
                   if t[3] != 0 or (p, t[0], t[1], t[2]) in pe_a0)
        n_tt = len(terms) - n_pe
        for e in range(NE):
            pick(("blk", gi, e), 1024, n_pe)         # eighth mult block
        for hf in range(2):
            for oh in range(2):
                pick(("ttc", gi, hf, oh), 2048, 2 * n_tt)  # TT chain (mult+add)
        for e in range(NE):
            for hb in range(2):
                if gi == len(PGROUPS) - 1:
                    assign[("fold", gi, e, hb)] = "v"
                    busy["v"] += C_DVE[512]
                else:
                    assign[("fold", gi, e, hb)] = "2s"
                    busy["g"] += C_POOL[512]
    return pe_a0, assign

PE_A0, ENG_ASSIGN = _plan()

_NC_CACHE = {}


def _build_nc():
    import concourse.bacc as bacc
    import concourse.mybir as mybir
    from concourse.tile import TileContext

    fp16 = mybir.dt.float16
    f32 = mybir.dt.float32
    AF = mybir.ActivationFunctionType
    OP = mybir.AluOpType

    nc = bacc.Bacc("TRN2", target_bir_lowering=False)

    x_in = nc.dram_tensor("x", [CIN, HW], f32, kind="ExternalInput")
    woff_in = nc.dram_tensor("woff", [128, len(OFF_MMS) * 32], fp16, kind="ExternalInput")
    boff_in = nc.dram_tensor("boff", [1, 512], fp16, kind="ExternalInput")
    wy_in = nc.dram_tensor("wy", [CIN, KK * 64], fp16, kind="ExternalInput")
    id_in = nc.dram_tensor("ident", [128, 132], fp16, kind="ExternalInput")
    out_t = nc.dram_tensor("out", [COUT, HW], f32, kind="ExternalOutput")

    def eng(key):
        return nc.vector if ENG_ASSIGN[key] == "v" else nc.gpsimd

    with TileContext(nc) as tc:
        with (
            tc.tile_pool(name="persist", bufs=1) as pp,
            tc.tile_pool(name="psum_y", bufs=2, space="PSUM") as ppy,
        ):
            # ---- persistent sbuf tensors ----
            # xpair: partitions 0:63 = padded x (fp16), 64:127 = same shifted
            # one column left (reads x[c, r, w+1] at the same window offset)
            xpair = pp.tile([128, XSZ], fp16, tag="xpair")
            woff_sb = pp.tile([128, len(OFF_MMS) * 32], fp16, tag="woff")
            wy_sb = pp.tile([CIN, KK * 64], fp16, tag="wy")
            boff_sb = pp.tile([1, 512], fp16, tag="boff")
            ones1 = pp.tile([1, 128], fp16, tag="ones1")
            nc.vector.memset(ones1[:], 1.0)
            u_t = {rr: pp.tile([128, KK * W], fp16, tag=f"u{ri}", name=f"u{ri}")
                   for ri, rr in enumerate(RYRX)}
            ush_t = {rr: pp.tile([128, 6 * W], fp16, tag=f"s{ri}", name=f"s{ri}")
                     for ri, rr in enumerate(RYRX)}
            Q = pp.tile([128, COUT * W], fp16, tag="q", name="q")
            i132 = pp.tile([128, 132], fp16, tag="i132")
            ident = i132[:, 2:130]
            cst = pp.tile([128, 3], f32, tag="cst")  # columns: -1.0, 0.0, +1.0
            nc.vector.memset(cst[:, 0:1], -1.0)
            nc.vector.memset(cst[:, 1:2], 0.0)
            nc.vector.memset(cst[:, 2:3], 1.0)
            cbias = {-1.0: cst[:, 0:1], 0.0: cst[:, 1:2], 1.0: cst[:, 2:3]}

            # ---- load constants ----
            nc.sync.dma_start(woff_sb[:], woff_in[:])
            nc.sync.dma_start(wy_sb[:], wy_in[:])
            nc.sync.dma_start(boff_sb[:], boff_in[:])
            nc.sync.dma_start(i132[:], id_in[:])

            # ---- load x into padded layout (f32 -> fp16 cast in DMA) ----
            xpr = xpair[:].rearrange("c (r w) -> c r w", w=XP)
            nc.gpsimd.memset(xpr[0:64, 0:1, :], 0.0)       # top pad row
            nc.gpsimd.memset(xpr[0:64, 129:130, :], 0.0)   # bottom pad row
            nc.gpsimd.memset(xpr[0:64, :, 0:1], 0.0)       # left pad col
            nc.gpsimd.memset(xpr[0:64, :, 129:130], 0.0)   # right pad col
            for ci in range(8):
                r0, r1 = 1 + ci * 16, 17 + ci * 16
                nc.gpsimd.dma_start(
                    xpr[0:64, r0:r1, 1:1 + W],
                    x_in[:, (r0 - 1) * W:(r1 - 1) * W].rearrange("c (r w) -> c r w", w=W),
                )
            # column-shifted copy on partitions 64:127 (SBUF->SBUF, no cast)
            nc.gpsimd.memset(xpair[64:128, XSZ - 1:XSZ], 0.0)
            bnds = (0,) + tuple((17 + 16 * ci) * XP for ci in range(7)) + (XSZ - 1,)
            for b0, b1 in zip(bnds[:-1], bnds[1:]):
                nc.sync.dma_start(
                    xpair[64:128, b0:b1],
                    xpair[0:64, b0 + 1:b1 + 1],
                )

            for rr in RYRX:
                nc.gpsimd.memset(ush_t[rr][:], 0.0)
            nc.gpsimd.memset(Q[:], 0.0)

            with (
                tc.tile_pool(name="yt", bufs=2) as pyt,
                tc.tile_pool(name="ysl", bufs=1) as pysl,
            ):
                yt_tiles = {}

                def produce_alloc(pi):
                    ks = list(PAIRS[pi])
                    for k in ks:
                        ytk = pyt.tile([128, COUT * WY], fp16, tag="yt",
                                       name=f"yt{k}", bufs=5)
                        yt_tiles[k] = ytk
                        ytr0 = ytk[:].rearrange("h (o w) -> h o w", w=WY)
                        nc.scalar.memzero(ytr0[:, :, 0:2])
                        nc.scalar.memzero(ytr0[:, :, WY - 2:WY])

                def produce_quarters(pi, whs, helper=False):
                    # Y computed directly in [h-part, (o, w)] layout: per w, a
                    # matmul with the x column as stationary:
                    #   psum[h, o] = sum_c x[c, (h, w)] * wy_k[c, o]
                    for wh in whs:               # w-quarters of 32 columns
                        for k in PAIRS[pi]:
                            rhsw = wy_sb[:, k * 64:(k + 1) * 64]
                            for wb in range(4):  # 8-w psum tiles
                                wa = wh * 32 + wb * 8
                                psum = ppy.tile([128, 8 * 64], f32, tag="psy",
                                                name="psy")
                                for wi in range(8):
                                    xcol = xpr[0:64, 1:129, 1 + wa + wi]
                                    nc.tensor.matmul(
                                        psum[:, wi * 64:(wi + 1) * 64],
                                        xcol, rhsw, start=True, stop=True)
                                dtile = yt_tiles[k][:].rearrange(
                                    "h (o w) -> h w o", o=COUT)[
                                    :, 2 + wa: 2 + wa + 8, :]
                                psrc = psum[:].rearrange("h (w o) -> h w o", o=64)
                                if helper and wb % 2 == 1:
                                    nc.vector.tensor_scalar(dtile, psrc, 0.0,
                                                            None, OP.add)
                                else:
                                    nc.scalar.activation(dtile, psrc, AF.Copy)

                # =========== phase 1: offset conv + tents + u fields ===========
                with (
                    tc.tile_pool(name="ph1", bufs=1) as p1,
                    tc.tile_pool(name="scr", bufs=2) as scr,
                    tc.tile_pool(name="psum_off", bufs=2, space="PSUM") as ppo,
                ):
                    # off_t layout: [h-partitions, (c32, w)] w-innermost
                    off_t = p1.tile([128, 32 * W], fp16, tag="offt")
                    offr = off_t[:].rearrange("h (c w) -> h c w", w=W)

                    produce_alloc(0)
                    # column-stationary offset conv: per output w, the x
                    # column is the matmul stationary, so psum lands directly
                    # transposed [h, (w, ch)]; taps accumulate per w-slot and
                    # one ones-row matmul adds the bias across the tile.
                    for t8 in range(8):
                        w0 = t8 * 16
                        psum = ppo.tile([128, 512], f32, tag="psoff")
                        for wi in range(16):
                            w = w0 + wi
                            sl = psum[:, wi * 32:(wi + 1) * 32]
                            nc.tensor.matmul(sl, ones1[:],
                                             boff_sb[:, wi * 32:(wi + 1) * 32],
                                             start=True, stop=False,
                                             skip_group_check=True)
                            for mi, (ks_mm, ki, c0) in enumerate(OFF_MMS):
                                nprt = 64 * len(ks_mm)
                                lhs = xpr[0:nprt, 1 + ki:129 + ki, c0 + w]
                                nc.tensor.matmul(
                                    sl, lhs,
                                    woff_sb[0:nprt, mi * 32:(mi + 1) * 32],
                                    start=False, stop=(mi == len(OFF_MMS) - 1),
                                    skip_group_check=True)
                        dst = offr[:, :, w0:w0 + 16]
                        psrc = psum[:].rearrange("h (w c) -> h c w", c=32)
                        if t8 % 2 == 0:
                            nc.vector.tensor_scalar(dst, psrc, 0.0, None, OP.add)
                        else:
                            nc.scalar.activation(dst, psrc, AF.Copy)
                            produce_quarters(0, [t8 // 2], helper=True)

                    # tents and u products, batched across all 9 kernel points
                    dy_all = offr[:, 0:9, :]
                    dx_all = offr[:, 9:18, :]
                    lg_all = offr[:, 18:27, :]
                    msk = p1.tile([128, KK * W], fp16, tag="msk")
                    mskr = msk[:].rearrange("h (k w) -> h k w", w=W)
                    nc.scalar.activation(mskr, lg_all, AF.Sigmoid, bias=cbias[0.0])
                    # tents via relu identities (DVE tensor_scalar runs at 4x):
                    # tent(d-1)=relu(d), tent(d+1)=relu(-d), tent(d)=1-relu(d)-relu(-d)
                    # y tents persist; x tents go through shared scratch and
                    # fold the mask in immediately
                    dy_f, dx_f = off_t[:, 0:9 * W], off_t[:, 9 * W:18 * W]
                    typ = scr.tile([128, KK * W], fp16, tag="typ", bufs=1)
                    nc.vector.tensor_scalar(typ[:], dy_f, 0.0, None, OP.max)
                    tyn = scr.tile([128, KK * W], fp16, tag="tyn", bufs=1)
                    nc.vector.tensor_scalar(tyn[:], dy_f, -1.0, 0.0, OP.mult, OP.max)
                    tsum = scr.tile([128, KK * W], fp16, tag="tscr", name="tscr",
                                    bufs=1)
                    nc.vector.tensor_tensor(tsum[:], typ[:], tyn[:], OP.add)
                    tyz = scr.tile([128, KK * W], fp16, tag="tyz", bufs=1)
                    nc.vector.tensor_scalar(tyz[:], tsum[:], -1.0, 1.0,
                                            OP.mult, OP.add)
                    ty = {1: typ, -1: tyn, 0: tyz}
                    txm = {}
                    txp = scr.tile([128, KK * W], fp16, tag="txsh", name="txsh",
                                   bufs=2)
                    nc.vector.tensor_scalar(txp[:], dx_f, 0.0, None, OP.max)
                    txn = scr.tile([128, KK * W], fp16, tag="txsh", name="txsh",
                                   bufs=2)
                    nc.vector.tensor_scalar(txn[:], dx_f, -1.0, 0.0, OP.mult, OP.max)
                    tsum2 = scr.tile([128, KK * W], fp16, tag="tscr", name="tscr",
                                     bufs=1)
                    nc.vector.tensor_tensor(tsum2[:], txp[:], txn[:], OP.add)
                    for r, tsrc in ((1, txp), (-1, txn)):
                        txmr = scr.tile([128, KK * W], fp16, tag=f"txm{r}", bufs=1)
                        nc.vector.tensor_tensor(txmr[:], tsrc[:], msk[:], OP.mult)
                        txm[r] = txmr
                    txz = scr.tile([128, KK * W], fp16, tag="txsh", name="txsh",
                                   bufs=2)
                    nc.vector.tensor_scalar(txz[:], tsum2[:], -1.0, 1.0,
                                            OP.mult, OP.add)
                    txm0 = scr.tile([128, KK * W], fp16, tag="txm0", bufs=1)
                    nc.vector.tensor_tensor(txm0[:], txz[:], msk[:], OP.mult)
                    txm[0] = txm0
                    for (ry, rx) in RYRX:
                        nc.vector.tensor_tensor(u_t[(ry, rx)][:], ty[ry][:],
                                                txm[rx][:], OP.mult)
                    # row-shifted copies per k-band (ki = band - 1), band-major
                    # so the first FMA group's bands land first; ush_t holds
                    # only the two ki != -ry bands
                    for bi, ki in enumerate((-1, 0, 1)):
                        for (ry, rx) in RYRX:
                            a = ki + ry
                            if a == 0:
                                continue
                            sidx = [kv for kv in (-1, 0, 1) if kv != -ry].index(ki)
                            sband = slice(sidx * 3 * W, (sidx + 1) * 3 * W)
                            band = slice(bi * 3 * W, (bi + 1) * 3 * W)
                            if a > 0:
                                nc.sync.dma_start(ush_t[(ry, rx)][a:128, sband],
                                                  u_t[(ry, rx)][0:128 - a, band])
                            else:
                                nc.sync.dma_start(ush_t[(ry, rx)][0:128 + a, sband],
                                                  u_t[(ry, rx)][-a:128, band])

                # =========== phase 2: remaining Y maps + FMA accumulation ===========
                qr = Q[:].rearrange("h (o w) -> h o w", w=W)
                with (
                    tc.tile_pool(name="fma_ps", bufs=4, space="PSUM") as ppq,
                    tc.tile_pool(name="ftmp", bufs=4) as ptmp,
                ):
                    for gi, grp in enumerate(PGROUPS):
                        if gi + 1 < len(PGROUPS):
                            for pn in PGROUPS[gi + 1]:
                                produce_alloc(pn)
                                produce_quarters(pn, range(4))
                        terms = [(p, t) for p in grp for t in _pair_terms(p)]
                        pe_terms = [t for p, t in terms
                                    if t[3] != 0 or (p, t[0], t[1], t[2]) in PE_A0]
                        tt_by_p = {p: [t for t in _pair_terms(p)
                                       if t[3] == 0 and
                                       (p, t[0], t[1], t[2]) not in PE_A0]
                                   for p in grp}

                        # TT chains: direct Q adds for unshifted terms, chunked
                        # (hf, oh) so each chain stays on one engine's queue
                        chains_by_eng = {"v": [], "g": []}
                        for hf in range(2):
                            for oh in range(2):
                                ekey = ENG_ASSIGN[("ttc", gi, hf, oh)]
                                tg = "vtt" if ekey == "v" else "gtt"
                                ops = []
                                for p in grp:
                                    for (k, ry, rx, a, ax) in tt_by_p[p]:
                                        ops.append((eng(("ttc", gi, hf, oh)), tg,
                                                    k, ry, rx, ax, hf, oh))
                                if ops:
                                    chains_by_eng[ekey].append(ops)

                        # PSUM-accumulated terms per w-eighth (one accumulation
                        # group spanning all pairs of grp), TT chains spread
                        # between eighth blocks on the opposite engine; one
                        # chain leads (it needs only u_t, not the ush copies)
                        pend = []
                        for ekey in ("v", "g"):
                            if chains_by_eng[ekey]:
                                pend.append(chains_by_eng[ekey].pop(0))
                        for chain in pend:
                            for (egn, tg, k, ry, rx, ax, hf, oh) in chain:
                                ytr = yt_tiles[k][:].rearrange(
                                    "h (o w) -> h o w", w=WY)
                                yr = ytr[:, oh * 32:(oh + 1) * 32,
                                         2 + ax + hf * 64: 2 + ax + hf * 64 + 64]
                                ub = u_t[(ry, rx)][:, k * W + hf * 64: k * W + hf * 64 + 64] \
                                    .rearrange("p (z w) -> p z w", z=1) \
                                    .broadcast_to([128, 32, 64])
                                tmp = ptmp.tile([128, 32 * 64], fp16, tag=tg,
                                                name=tg, bufs=1)
                                tr = tmp[:].rearrange("p (o w) -> p o w", w=64)
                                egn.tensor_tensor(tr, yr, ub, OP.mult)
                                qs = qr[:, oh * 32:(oh + 1) * 32,
                                        hf * 64:(hf + 1) * 64]
                                egn.tensor_tensor(qs, qs, tr, OP.add)
                        for e in range(NE):
                            w0 = e * EW
                            blk_eng = eng(("blk", gi, e))
                            blk_tag = "vtmp" if ENG_ASSIGN[("blk", gi, e)] == "v" else "gtmp"
                            pbank = [ppq.tile([128, 512], f32, tag=f"psq{hb}",
                                              name=f"psq{hb}", bufs=3) for hb in range(2)]
                            for ti, (k, ry, rx, a, ax) in enumerate(pe_terms):
                                ki = k // 3 - 1
                                if a == 0:
                                    usrc, kcol = u_t[(ry, rx)], k
                                else:
                                    sidx = [kv for kv in (-1, 0, 1)
                                            if kv != -ry].index(ki)
                                    usrc, kcol = ush_t[(ry, rx)], sidx * 3 + k % 3
                                ytr = yt_tiles[k][:].rearrange("h (o w) -> h o w", w=WY)
                                yr = ytr[:, :, 2 + ax + w0: 2 + ax + w0 + EW]
                                ub = usrc[:, kcol * W + w0: kcol * W + w0 + EW] \
                                    .rearrange("p (z w) -> p z w", z=1) \
                                    .broadcast_to([128, 64, EW])
                                tmp = ptmp.tile([128, 64 * EW], fp16, tag=blk_tag,
                                                name=blk_tag, bufs=6)
                                tr = tmp[:].rearrange("p (o w) -> p o w", w=EW)
                                blk_eng.tensor_tensor(tr, yr, ub, OP.mult)
                                sa = i132[:, 2 + a:2 + a + 128]
                                st = (ti == 0)
                                sp = (ti == len(pe_terms) - 1)
                                for hb in range(2):
                                    nc.tensor.matmul(
                                        pbank[hb][:], sa, tmp[:, hb * 512:(hb + 1) * 512],
                                        start=st, stop=sp)
                            for hb in range(2):
                                qs = qr[:, hb * 32:(hb + 1) * 32, w0:w0 + EW]
                                pr_ap = pbank[hb][:].rearrange(
                                    "h (o w) -> h o w", w=EW)
                                if ENG_ASSIGN[("fold", gi, e, hb)] == "2s":
                                    stg = ptmp.tile([128, 512], fp16, tag="fstg",
                                                    name="fstg", bufs=2)
                                    sr = stg[:].rearrange("h (o w) -> h o w", w=EW)
                                    nc.scalar.activation(sr, pr_ap, AF.Copy)
                                    nc.gpsimd.tensor_tensor(qs, qs, sr, OP.add)
                                else:
                                    eng(("fold", gi, e, hb)).tensor_tensor(
                                        qs, qs, pr_ap, OP.add)
                            # interleave one TT chain after every other eighth
                            if e % 2 == 1:
                                nxt = ENG_ASSIGN[("blk", gi, e + 1)] if e + 1 < NE else "v"
                                opp = "g" if nxt == "v" else "v"
                                chain = (chains_by_eng[opp].pop(0)
                                         if chains_by_eng[opp]
                                         else (chains_by_eng[nxt].pop(0)
                                               if chains_by_eng[nxt] else None))
                                if chain:
                                    for (egn, tg, k, ry, rx, ax, hf, oh) in chain:
                                        ytr = yt_tiles[k][:].rearrange(
                                            "h (o w) -> h o w", w=WY)
                                        yr = ytr[:, oh * 32:(oh + 1) * 32,
                                                 2 + ax + hf * 64: 2 + ax + hf * 64 + 64]
                                        ub = u_t[(ry, rx)][:, k * W + hf * 64: k * W + hf * 64 + 64] \
                                            .rearrange("p (z w) -> p z w", z=1) \
                                            .broadcast_to([128, 32, 64])
                                        tmp = ptmp.tile([128, 32 * 64], fp16, tag=tg,
                                                        name=tg, bufs=1)
                                        tr = tmp[:].rearrange("p (o w) -> p o w", w=64)
                                        egn.tensor_tensor(tr, yr, ub, OP.mult)
                                        qs = qr[:, oh * 32:(oh + 1) * 32,
                                                hf * 64:(hf + 1) * 64]
                                        egn.tensor_tensor(qs, qs, tr, OP.add)
                        for p in grp:
                            for k in PAIRS[p]:
                                yt_tiles.pop(k)

                    # ---- write halves ----
                    dst_f = out_t[:].rearrange("o (h w) -> h o w", w=W)
                    for hf in range(2):
                        osl = slice(hf * 32, (hf + 1) * 32)
                        nc.gpsimd.dma_start(dst_f[:, osl, :], qr[:, osl, :])

    nc.compile()
    return nc


def _prep_weights(w_off, b_off, w_dcn):
    perm = list(range(0, 17, 2)) + list(range(1, 18, 2)) + list(range(18, 27))
    w_off_p = w_off[perm]          # [27, 64, 3, 3] rows = dy(9), dx(9), logit(9)
    b_off_p = b_off[perm]
    # paired-tap weight packing: [128 partitions, n_mm * 32]
    woff_host = np.zeros((128, len(OFF_MMS) * 32), np.float16)
    for mi, (ks_mm, _ki, _c0) in enumerate(OFF_MMS):
        for j, k in enumerate(ks_mm):
            kyi, kxi = k // 3, k % 3
            woff_host[j * 64:(j + 1) * 64, mi * 32:mi * 32 + 27] = \
                w_off_p[:, :, kyi, kxi].T.astype(np.float16)
    b32 = np.zeros(32, np.float32)
    b32[:27] = b_off_p
    boff_host = np.tile(b32, 16).astype(np.float16).reshape(1, 512)
    wdr = w_dcn.reshape(COUT, CIN, KK)
    wy_host = np.zeros((KK, CIN, 64), np.float16)
    for k in range(KK):
        wy_host[k, :, :] = wdr[:, :, k].T.astype(np.float16)
    wy_host = np.ascontiguousarray(wy_host.transpose(1, 0, 2).reshape(CIN, KK * 64))
    ident_host = np.zeros((128, 132), np.float16)
    for p in range(128):
        ident_host[p, p + 2] = 1.0
    return woff_host, boff_host, wy_host, ident_host


def kernel(x, w_off, b_off, w_dcn):
    from concourse.bass_utils import run_bass_kernel_spmd

    if "nc" not in _NC_CACHE:
        _NC_CACHE["nc"] = _build_nc()
    nc = _NC_CACHE["nc"]

    woff_host, boff_host, wy_host, ident_host = _prep_weights(
        np.asarray(w_off, np.float32), np.asarray(b_off, np.float32),
        np.asarray(w_dcn, np.float32))
    x = np.asarray(x, np.float32)
    in_maps = [{
        "x": np.ascontiguousarray(x[b].reshape(CIN, HW)),
        "woff": woff_host, "boff": boff_host, "wy": wy_host, "ident": ident_host,
    } for b in range(B)]
    import os
    import time
    os.environ.setdefault("BASS_NEVER_TRACE", "1")
    res = None
    for attempt in range(3):
        try:
            res = run_bass_kernel_spmd(nc, in_maps, core_ids=list(range(B)))
            break
        except Exception:
            # transient NRT device errors clear on retry
            if attempt == 2:
                raise
            time.sleep(10)
    _NC_CACHE["last_results"] = res
    out = np.stack([res.results[b]["out"].reshape(COUT, H, W) for b in range(B)])
    out = out.astype(np.float32)
    _fixup_large_offsets(out, x, np.asarray(w_off, np.float32),
                         np.asarray(b_off, np.float32), np.asarray(w_dcn, np.float32))
    return out


def _fixup_large_offsets(out, x, w_off, b_off, w_dcn):
    """The on-device kernel uses a 3-tap tent decomposition of the bilinear
    interpolation, exact only for |offset| < 1. Offsets exceed 1 at ~1e-4 of
    sample points; recompute those output pixels exactly on host."""
    perm = list(range(0, 17, 2)) + list(range(1, 18, 2)) + list(range(18, 27))
    w_p = w_off[perm]
    b_p = b_off[perm]
    xpad = np.zeros((B, CIN, H + 2, W + 2), np.float32)
    xpad[:, :, 1:-1, 1:-1] = x
    off = np.zeros((B, 27, H, W), np.float32)
    for k in range(KK):
        kyi, kxi = k // 3, k % 3
        off += np.einsum("mc,bchw->bmhw", w_p[:, :, kyi, kxi],
                         xpad[:, :, kyi:kyi + H, kxi:kxi + W])
    off += b_p[None, :, None, None]
    dy, dx, lg = off[:, :9], off[:, 9:18], off[:, 18:27]
    bad = ((np.abs(dy) > 0.998) | (np.abs(dx) > 0.998)).any(axis=1)  # [B, H, W]
    if not bad.any():
        return
    wdr = w_dcn.reshape(COUT, CIN, KK)
    mask_all = 1.0 / (1.0 + np.exp(-lg))
    for b, h, w in zip(*np.nonzero(bad)):
        val = np.zeros((CIN, KK), np.float32)
        for k in range(KK):
            ki, kj = k // 3 - 1, k % 3 - 1
            py = h + ki + dy[b, k, h, w]
            px = w + kj + dx[b, k, h, w]
            y0, x0 = int(np.floor(py)), int(np.floor(px))
            wy1, wx1 = py - y0, px - x0
            acc = np.zeros(CIN, np.float32)
            for (yy, wyv) in ((y0, 1 - wy1), (y0 + 1, wy1)):
                for (xx, wxv) in ((x0, 1 - wx1), (x0 + 1, wx1)):
                    if 0 <= yy < H and 0 <= xx < W:
                        acc += np.float32(wyv * wxv) * x[b, :, yy, xx]
            val[:, k] = acc * mask_all[b, k, h, w]
        out[b, :, h, w] = np.einsum("ock,ck->o", wdr, val)



# revision 4
# speedup vs baseline: 1.0626x; 1.0626x over previous
"""DeformConv2d Bass kernel for trn2 (8 NeuronCores, batch-sharded).

Algorithm (per core, one image, fp16 compute):
  1. offset conv (PE): off[27, HW] = sum_k Woff_k @ x_shift_k + b, with taps
     paired on the contraction dim (x + a column-shifted copy of x stacked on
     partitions 64:127) -> 6 matmuls per psum tile instead of 9.
  2. Y_k = W_dcn[:,:,k] @ x for the 9 kernel points, PE-transposed to
     [h-partitions, (o, w)] tiles (ACT drains).
  3. bilinear interp as dense 3-tap tent product:
       out[o,h,w] = sum_k sum_{ry,rx} u_{k,ry,rx}[h,w] * Y_k[o, h+ki+ry, w+kj+rx]
     u = sigmoid(logit) * tent(dy-ry) * tent(dx-rx), exact for |dy|,|dx| < 1.
  4. every term goes through f32 PSUM accumulation on the PE (shifted-identity
     matmuls); per-pixel products run on DVE (fp16) and Pool (fp8 out).
     Pool-made fp8 products are paired two-at-a-time into fp8 DoubleRow
     matmuls (2x PE throughput); the 9 dominant center-tap terms stay on the
     fp16 path so fp8 rounding only touches terms ~50x smaller.
     3 k-groups of 3, fold PSUM into the fp16 accumulator Q once per
     (group, w-eighth, o-half).
"""

import numpy as np

B, CIN, COUT, H, W, K, PAD = 8, 64, 64, 128, 128, 3, 1
KK = K * K
HW = H * W            # 16384
XP = 130              # padded x row stride / rows
XSZ = XP * XP         # padded x elements per partition
WY = W + 4            # padded w-stride in transposed Y: 132 (w in -2..129)
KGROUPS = [(0, 1, 2), (3, 4, 5), (6, 7, 8)]
NE = 8                # FMA w-eighths
EW = W // NE          # 16 w-cols per eighth

# offset-conv tap pairing: within each ki row, (kj=-1, kj=0) share a matmul
# via the column-shifted x copy; kj=+1 runs alone on partitions 0:63.
OFF_MMS = []
for _ki in (-1, 0, 1):
    OFF_MMS.append(([3 * (_ki + 1) + 2], _ki, 2))                     # kj=+1
for _ki in (-1, 0, 1):
    OFF_MMS.append(([3 * (_ki + 1) + 0, 3 * (_ki + 1) + 1], _ki, 0))  # kj=-1 & kj=0

RYRX = [(ry, rx) for ry in (-1, 0, 1) for rx in (-1, 0, 1)]


def _terms(k):
    ki, kj = k // 3 - 1, k % 3 - 1
    return [(k, ry, rx, ki + ry, kj + rx) for (ry, rx) in RYRX]


# ---- static engine / dtype plan --------------------------------------------
# Every term goes through PSUM on the PE.  The product u*Y runs on DVE
# (fp16 out, 2x mode) or Pool (fp8 out, same cost as fp16 on Pool); Pool/fp8
# terms pair into DoubleRow matmuls at half PE cost.  The (ry,rx)=(0,0)
# center terms carry ~95% of the output variance -> force fp16/DVE.
CV, CG = 4752.0, 6824.0          # per-term product cost (8 eighths)
FV, FG = 658.0, 427.0            # per-fold cost


def _plan():
    busy = {"v": 18000.0, "g": 4000.0}
    assign = {}
    for ks in KGROUPS:
        terms = [t for k in ks for t in _terms(k)]
        # centers first (forced v), then the rest by greedy balance;
        # corners (smallest u) preferentially to fp8.
        for t in terms:
            if t[1] == 0 and t[2] == 0:
                assign[(t[0], t[1], t[2])] = "v"
                busy["v"] += CV
        rest = [t for t in terms if not (t[1] == 0 and t[2] == 0)]
        rest.sort(key=lambda t: -(abs(t[1]) + abs(t[2])))  # corners first
        for t in rest:
            if busy["g"] + CG <= busy["v"] + CV:
                assign[(t[0], t[1], t[2])] = "g"
                busy["g"] += CG
            else:
                assign[(t[0], t[1], t[2])] = "v"
                busy["v"] += CV
    folds = {}
    for gi in range(len(KGROUPS)):
        for e in range(NE):
            for hb in range(2):
                if busy["g"] + FG <= busy["v"] + FV:
                    folds[(gi, e, hb)] = "g"
                    busy["g"] += FG
                else:
                    folds[(gi, e, hb)] = "v"
                    busy["v"] += FV
    return assign, folds


ASSIGN, FOLD_ASSIGN = _plan()


def _group_pairs(ks):
    """Pool/fp8 terms of a group -> DR pairs (t0, t1) + singles, preferring
    same-shift pairs. Returns (pairs [(t0, t1)], singles [t])."""
    gterms = [t for k in ks for t in _terms(k)
              if ASSIGN[(t[0], t[1], t[2])] == "g"]
    by_a = {}
    for t in gterms:
        by_a.setdefault(t[3], []).append(t)
    pairs, leftover = [], []
    for a in sorted(by_a):
        lst = by_a[a]
        while len(lst) >= 2:
            pairs.append((lst.pop(), lst.pop()))
        leftover.extend(lst)
    while len(leftover) >= 2:
        pairs.append((leftover.pop(), leftover.pop()))
    return pairs, leftover


_PACKS = []          # list of (a0, a1) for DR ident packs
_PACK_IDX = {}
for _ks in KGROUPS:
    for _p in _group_pairs(_ks)[0]:
        _key = (_p[0][3], _p[1][3])
        if _key not in _PACK_IDX:
            _PACK_IDX[_key] = len(_PACKS)
            _PACKS.append(_key)

_NC_CACHE = {}


def _build_nc():
    import concourse.bacc as bacc
    import concourse.mybir as mybir
    from concourse.tile import TileContext

    fp16 = mybir.dt.float16
    fp8 = mybir.dt.float8e4
    f32 = mybir.dt.float32
    AF = mybir.ActivationFunctionType
    OP = mybir.AluOpType
    DR = mybir.MatmulPerfMode.DoubleRow

    nc = bacc.Bacc("TRN2", target_bir_lowering=False)

    x_in = nc.dram_tensor("x", [CIN, HW], fp16, kind="ExternalInput")
    woff_in = nc.dram_tensor("woff", [128, len(OFF_MMS) * 32], fp16, kind="ExternalInput")
    boff_in = nc.dram_tensor("boff", [1, 512], fp16, kind="ExternalInput")
    wy_in = nc.dram_tensor("wy", [CIN, KK * 64], fp16, kind="ExternalInput")
    id_in = nc.dram_tensor("ident", [128, 132], fp16, kind="ExternalInput")
    # fp8 identity blob: 5 plain shifted idents (a=-2..2) + DR pair packs
    id8_in = nc.dram_tensor("ident8", [128, (5 + 2 * len(_PACKS)) * 128], f32,
                            kind="ExternalInput")
    out_t = nc.dram_tensor("out", [COUT, HW], fp16, kind="ExternalOutput")

    def eng(key, table):
        return nc.vector if table[key] == "v" else nc.gpsimd

    with TileContext(nc) as tc:
        with (
            tc.tile_pool(name="persist", bufs=1) as pp,
            tc.tile_pool(name="psum_y", bufs=2, space="PSUM") as ppy,
        ):
            # ---- persistent sbuf tensors ----
            xpair = pp.tile([128, XSZ], fp16, tag="xpair")
            woff_sb = pp.tile([128, len(OFF_MMS) * 32], fp16, tag="woff")
            wy_sb = pp.tile([CIN, KK * 64], fp16, tag="wy")
            boff_sb = pp.tile([1, 512], fp16, tag="boff")
            ones1 = pp.tile([1, 128], fp16, tag="ones1")
            nc.vector.memset(ones1[:], 1.0)
            u_t = {rr: pp.tile([128, KK * W], fp16, tag=f"u{ri}", name=f"u{ri}")
                   for ri, rr in enumerate(RYRX)}
            ush_t = {rr: pp.tile([128, 6 * W], fp16, tag=f"s{ri}", name=f"s{ri}")
                     for ri, rr in enumerate(RYRX)}
            Q = pp.tile([128, COUT * W], fp16, tag="q", name="q")
            i132 = pp.tile([128, 132], fp16, tag="i132")
            id8 = pp.tile([128, (5 + 2 * len(_PACKS)) * 128], fp8, tag="id8")
            cst = pp.tile([128, 3], f32, tag="cst")  # columns: -1.0, 0.0, +1.0
            nc.vector.memset(cst[:, 0:1], -1.0)
            nc.vector.memset(cst[:, 1:2], 0.0)
            nc.vector.memset(cst[:, 2:3], 1.0)
            cbias = {-1.0: cst[:, 0:1], 0.0: cst[:, 1:2], 1.0: cst[:, 2:3]}

            def ident16(a):
                return i132[:, 2 + a:2 + a + 128]

            def ident8_plain(a):
                return id8[:, (a + 2) * 128:(a + 3) * 128]

            def ident8_pack(pi):
                base = (5 + 2 * pi) * 128
                return id8[:, base:base + 256].rearrange(
                    "p (t m) -> p t m", t=2)

            # ---- load constants ----
            nc.sync.dma_start(woff_sb[:], woff_in[:])
            nc.sync.dma_start(wy_sb[:], wy_in[:])
            nc.sync.dma_start(boff_sb[:], boff_in[:])
            nc.sync.dma_start(i132[:], id_in[:])
            nc.gpsimd.dma_start(id8[:], id8_in[:])   # cast f32 -> fp8

            # ---- load x into padded layout (fp16 from host) ----
            xpr = xpair[:].rearrange("c (r w) -> c r w", w=XP)
            nc.scalar.memzero(xpr[0:64, 0:1, :])       # top pad row
            nc.scalar.memzero(xpr[0:64, 129:130, :])   # bottom pad row
            nc.vector.memset(xpr[0:64, :, 0:1], 0.0)   # left pad col
            nc.vector.memset(xpr[0:64, :, 129:130], 0.0)  # right pad col
            for ci in range(8):
                r0, r1 = 1 + ci * 16, 17 + ci * 16
                nc.sync.dma_start(
                    xpr[0:64, r0:r1, 1:1 + W],
                    x_in[:, (r0 - 1) * W:(r1 - 1) * W].rearrange("c (r w) -> c r w", w=W),
                )
            # column-shifted copy on partitions 64:127 (SBUF->SBUF)
            nc.vector.memset(xpair[64:128, XSZ - 1:XSZ], 0.0)
            bnds = (0,) + tuple((17 + 16 * ci) * XP for ci in range(7)) + (XSZ - 1,)
            for b0, b1 in zip(bnds[:-1], bnds[1:]):
                nc.sync.dma_start(
                    xpair[64:128, b0:b1],
                    xpair[0:64, b0 + 1:b1 + 1],
                )

            for rr in RYRX:
                nc.scalar.memzero(ush_t[rr][:])
            nc.scalar.memzero(Q[:])

            with (
                tc.tile_pool(name="yt", bufs=2) as pyt,
            ):
                yt_tiles = {}

                def produce_alloc(ks):
                    for k in ks:
                        ytk = pyt.tile([128, COUT * WY], fp16, tag="yt",
                                       name=f"yt{k}", bufs=5)
                        yt_tiles[k] = ytk
                        ytr0 = ytk[:].rearrange("h (o w) -> h o w", w=WY)
                        nc.scalar.memzero(ytr0[:, :, 0:2])
                        nc.scalar.memzero(ytr0[:, :, WY - 2:WY])

                def produce_quarter(k, wh):
                    # Y computed directly in [h-part, (o, w)] layout: per w, a
                    # matmul with the x column as stationary:
                    #   psum[h, o] = sum_c x[c, (h, w)] * wy_k[c, o]
                    rhsw = wy_sb[:, k * 64:(k + 1) * 64]
                    for wb in range(4):  # 8-w psum tiles
                        wa = wh * 32 + wb * 8
                        psum = ppy.tile([128, 8 * 64], f32, tag="psy",
                                        name="psy")
                        for wi in range(8):
                            xcol = xpr[0:64, 1:129, 1 + wa + wi]
                            nc.tensor.matmul(
                                psum[:, wi * 64:(wi + 1) * 64],
                                xcol, rhsw, start=True, stop=True)
                        dtile = yt_tiles[k][:].rearrange(
                            "h (o w) -> h w o", o=COUT)[
                            :, 2 + wa: 2 + wa + 8, :]
                        psrc = psum[:].rearrange("h (w o) -> h w o", o=64)
                        nc.scalar.activation(dtile, psrc, AF.Copy)

                # =========== phase 1: offset conv + tents + u fields ===========
                with (
                    tc.tile_pool(name="ph1", bufs=1) as p1,
                    tc.tile_pool(name="scr", bufs=2) as scr,
                    tc.tile_pool(name="psum_off", bufs=2, space="PSUM") as ppo,
                ):
                    # off_t layout: [h-partitions, (c32, w)] w-innermost
                    off_t = p1.tile([128, 32 * W], fp16, tag="offt")
                    offr = off_t[:].rearrange("h (c w) -> h c w", w=W)

                    produce_alloc(KGROUPS[0])
                    # column-stationary offset conv: per output w, the x
                    # column is the matmul stationary, so psum lands directly
                    # transposed [h, (w, ch)]; taps accumulate per w-slot and
                    # one ones-row matmul adds the bias across the tile.
                    for t8 in range(8):
                        w0 = t8 * 16
                        psum = ppo.tile([128, 512], f32, tag="psoff")
                        for wi in range(16):
                            w = w0 + wi
                            sl = psum[:, wi * 32:(wi + 1) * 32]
                            nc.tensor.matmul(sl, ones1[:],
                                             boff_sb[:, wi * 32:(wi + 1) * 32],
                                             start=True, stop=False,
                                             skip_group_check=True)
                            for mi, (ks_mm, ki, c0) in enumerate(OFF_MMS):
                                nprt = 64 * len(ks_mm)
                                lhs = xpr[0:nprt, 1 + ki:129 + ki, c0 + w]
                                nc.tensor.matmul(
                                    sl, lhs,
                                    woff_sb[0:nprt, mi * 32:(mi + 1) * 32],
                                    start=False, stop=(mi == len(OFF_MMS) - 1),
                                    skip_group_check=True)
                        dst = offr[:, :, w0:w0 + 16]
                        psrc = psum[:].rearrange("h (w c) -> h c w", c=32)
                        if t8 % 2 == 0:
                            nc.vector.tensor_scalar(dst, psrc, 0.0, None, OP.add)
                        else:
                            nc.scalar.activation(dst, psrc, AF.Copy)
                            # interleave group-0 Y production (3 quarters per slot)
                            for k in KGROUPS[0]:
                                produce_quarter(k, t8 // 2)

                    # tents and u products, batched across all 9 kernel points
                    dy_all = offr[:, 0:9, :]
                    dx_all = offr[:, 9:18, :]
                    lg_all = offr[:, 18:27, :]
                    msk = p1.tile([128, KK * W], fp16, tag="msk")
                    mskr = msk[:].rearrange("h (k w) -> h k w", w=W)
                    nc.scalar.activation(mskr, lg_all, AF.Sigmoid, bias=cbias[0.0])
                    # tents via relu identities (DVE tensor_scalar runs at 4x):
                    # tent(d-1)=relu(d), tent(d+1)=relu(-d), tent(d)=1-relu(d)-relu(-d)
                    dy_f, dx_f = off_t[:, 0:9 * W], off_t[:, 9 * W:18 * W]
                    typ = scr.tile([128, KK * W], fp16, tag="typ", bufs=1)
                    nc.vector.tensor_scalar(typ[:], dy_f, 0.0, None, OP.max)
                    tyn = scr.tile([128, KK * W], fp16, tag="tyn", bufs=1)
                    nc.vector.tensor_scalar(tyn[:], dy_f, -1.0, 0.0, OP.mult, OP.max)
                    tsum = scr.tile([128, KK * W], fp16, tag="tscr", name="tscr",
                                    bufs=1)
                    nc.vector.tensor_tensor(tsum[:], typ[:], tyn[:], OP.add)
                    tyz = scr.tile([128, KK * W], fp16, tag="tyz", bufs=1)
                    nc.vector.tensor_scalar(tyz[:], tsum[:], -1.0, 1.0,
                                            OP.mult, OP.add)
                    ty = {1: typ, -1: tyn, 0: tyz}
                    txm = {}
                    txp = scr.tile([128, KK * W], fp16, tag="txsh", name="txsh",
                                   bufs=2)
                    nc.vector.tensor_scalar(txp[:], dx_f, 0.0, None, OP.max)
                    txn = scr.tile([128, KK * W], fp16, tag="txsh", name="txsh",
                                   bufs=2)
                    nc.vector.tensor_scalar(txn[:], dx_f, -1.0, 0.0, OP.mult, OP.max)
                    tsum2 = scr.tile([128, KK * W], fp16, tag="tscr", name="tscr",
                                     bufs=1)
                    nc.vector.tensor_tensor(tsum2[:], txp[:], txn[:], OP.add)
                    for r, tsrc in ((1, txp), (-1, txn)):
                        txmr = scr.tile([128, KK * W], fp16, tag=f"txm{r}", bufs=1)
                        nc.vector.tensor_tensor(txmr[:], tsrc[:], msk[:], OP.mult)
                        txm[r] = txmr
                    txz = scr.tile([128, KK * W], fp16, tag="txsh", name="txsh",
                                   bufs=2)
                    nc.vector.tensor_scalar(txz[:], tsum2[:], -1.0, 1.0,
                                            OP.mult, OP.add)
                    txm0 = scr.tile([128, KK * W], fp16, tag="txm0", bufs=1)
                    nc.vector.tensor_tensor(txm0[:], txz[:], msk[:], OP.mult)
                    txm[0] = txm0
                    for (ry, rx) in RYRX:
                        nc.vector.tensor_tensor(u_t[(ry, rx)][:], ty[ry][:],
                                                txm[rx][:], OP.mult)
                    # row-shifted copies per k-band (ki = band - 1), band-major
                    # so the first group's bands land first; ush_t holds
                    # only the two ki != -ry bands
                    for bi, ki in enumerate((-1, 0, 1)):
                        for (ry, rx) in RYRX:
                            a = ki + ry
                            if a == 0:
                                continue
                            sidx = [kv for kv in (-1, 0, 1) if kv != -ry].index(ki)
                            sband = slice(sidx * 3 * W, (sidx + 1) * 3 * W)
                            band = slice(bi * 3 * W, (bi + 1) * 3 * W)
                            if a > 0:
                                nc.sync.dma_start(ush_t[(ry, rx)][a:128, sband],
                                                  u_t[(ry, rx)][0:128 - a, band])
                            else:
                                nc.sync.dma_start(ush_t[(ry, rx)][0:128 + a, sband],
                                                  u_t[(ry, rx)][-a:128, band])

                # =========== phase 2: remaining Y maps + FMA accumulation ===========
                qr = Q[:].rearrange("h (o w) -> h o w", w=W)

                def u_ap(t, w0, wn):
                    """u-field slice for a term, [128, 1, wn] broadcastable."""
                    k, ry, rx, a, b = t
                    ki = k // 3 - 1
                    if a == 0:
                        usrc, kcol = u_t[(ry, rx)], k
                    else:
                        sidx = [kv for kv in (-1, 0, 1) if kv != -ry].index(ki)
                        usrc, kcol = ush_t[(ry, rx)], sidx * 3 + k % 3
                    return usrc[:, kcol * W + w0: kcol * W + w0 + wn] \
                        .rearrange("p (z w) -> p z w", z=1)

                def y_ap(t, w0, wn):
                    k, ry, rx, a, b = t
                    ytr = yt_tiles[k][:].rearrange("h (o w) -> h o w", w=WY)
                    return ytr[:, :, 2 + b + w0: 2 + b + w0 + wn]

                with (
                    tc.tile_pool(name="fma_ps", bufs=4, space="PSUM") as ppq,
                    tc.tile_pool(name="ftmp", bufs=4) as ptmp,
                ):
                    for gi, ks in enumerate(KGROUPS):
                        v_terms = [t for k in ks for t in _terms(k)
                                   if ASSIGN[(t[0], t[1], t[2])] == "v"]
                        pairs, singles = _group_pairs(ks)
                        # macro-op sequence, v/g interleaved
                        g_units = [("p",) + p for p in pairs] + \
                                  [("s", t) for t in singles]
                        units = []
                        nv, ng = len(v_terms), len(g_units)
                        iv = ig = 0
                        for ui in range(nv + ng):
                            # proportional interleave
                            if iv * ng <= ig * nv and iv < nv:
                                units.append(("v", v_terms[iv])); iv += 1
                            elif ig < ng:
                                units.append(g_units[ig]); ig += 1
                            else:
                                units.append(("v", v_terms[iv])); iv += 1
                        # matmul count per eighth (for start/stop bookkeeping)
                        n_mm = sum(1 for u in units)

                        # production units for next group (spread over eighths)
                        prod_units = ([(k, wh) for wh in range(4)
                                       for k in KGROUPS[gi + 1]]
                                      if gi + 1 < len(KGROUPS) else [])
                        if gi + 1 < len(KGROUPS):
                            produce_alloc(KGROUPS[gi + 1])
                        pi_done = 0

                        for e in range(NE):
                            w0 = e * EW
                            pbank = [ppq.tile([128, 512], f32, tag=f"psq{hb}",
                                              name=f"psq{hb}", bufs=3)
                                     for hb in range(2)]
                            mm_i = 0
                            for unit in units:
                                st = (mm_i == 0)
                                sp = (mm_i == n_mm - 1)
                                if unit[0] == "v":
                                    t = unit[1]
                                    tmp = ptmp.tile([128, 1024], fp16,
                                                    tag="vtmp", name="vtmp",
                                                    bufs=6)
                                    tr = tmp[:].rearrange("p (o w) -> p o w",
                                                          w=EW)
                                    nc.vector.tensor_tensor(
                                        tr, y_ap(t, w0, EW),
                                        u_ap(t, w0, EW).broadcast_to(
                                            [128, 64, EW]), OP.mult)
                                    for hb in range(2):
                                        nc.tensor.matmul(
                                            pbank[hb][:], ident16(t[3]),
                                            tmp[:, hb * 512:(hb + 1) * 512],
                                            start=st, stop=sp)
                                elif unit[0] == "p":
                                    t0, t1 = unit[1], unit[2]
                                    t8 = ptmp.tile([128, 2048], fp8,
                                                   tag="gtmp", name="gtmp",
                                                   bufs=6)
                                    t8r = t8[:].rearrange(
                                        "p (t o w) -> p t o w", t=2, w=EW)
                                    for ti, t in ((0, t0), (1, t1)):
                                        nc.gpsimd.tensor_tensor(
                                            t8r[:, ti], y_ap(t, w0, EW),
                                            u_ap(t, w0, EW).broadcast_to(
                                                [128, 64, EW]), OP.mult)
                                    pk = ident8_pack(
                                        _PACK_IDX[(t0[3], t1[3])])
                                    mv = t8[:].rearrange(
                                        "p (t x) -> p t x", t=2)
                                    for hb in range(2):
                                        nc.tensor.matmul(
                                            pbank[hb][:], pk,
                                            mv[:, :, hb * 512:(hb + 1) * 512],
                                            start=st, stop=sp, perf_mode=DR)
                                else:
                                    t = unit[1]
                                    t8 = ptmp.tile([128, 2048], fp8,
                                                   tag="gtmp", name="gtmp",
                                                   bufs=6)
                                    t8r = t8[:].rearrange(
                                        "p (t o w) -> p t o w", t=2, w=EW)
                                    nc.gpsimd.tensor_tensor(
                                        t8r[:, 0], y_ap(t, w0, EW),
                                        u_ap(t, w0, EW).broadcast_to(
                                            [128, 64, EW]), OP.mult)
                                    for hb in range(2):
                                        nc.tensor.matmul(
                                            pbank[hb][:], ident8_plain(t[3]),
                                            t8[:, hb * 512:(hb + 1) * 512],
                                            start=st, stop=sp)
                                mm_i += 1
                            # fold PSUM into Q (Pool can't read PSUM: stage
                            # through ACT for "g" folds)
                            for hb in range(2):
                                qs = qr[:, hb * 32:(hb + 1) * 32, w0:w0 + EW]
                                pr_ap = pbank[hb][:].rearrange(
                                    "h (o w) -> h o w", w=EW)
                                if FOLD_ASSIGN[(gi, e, hb)] == "g":
                                    stg = ptmp.tile([128, 512], fp16,
                                                    tag="fstg", name="fstg",
                                                    bufs=3)
                                    sr = stg[:].rearrange(
                                        "h (o w) -> h o w", w=EW)
                                    nc.scalar.activation(sr, pr_ap, AF.Copy)
                                    nc.gpsimd.tensor_tensor(qs, qs, sr, OP.add)
                                else:
                                    nc.vector.tensor_tensor(qs, qs, pr_ap,
                                                            OP.add)
                            # interleave next-group production
                            tgt = (e + 1) * len(prod_units) // NE
                            while pi_done < tgt:
                                produce_quarter(*prod_units[pi_done])
                                pi_done += 1
                        for k in ks:
                            yt_tiles.pop(k)

                    # ---- write halves ----
                    dst_f = out_t[:].rearrange("o (h w) -> h o w", w=W)
                    for hf in range(2):
                        osl = slice(hf * 32, (hf + 1) * 32)
                        nc.sync.dma_start(dst_f[:, osl, :], qr[:, osl, :])

    nc.compile()
    return nc


def _prep_weights(w_off, b_off, w_dcn):
    perm = list(range(0, 17, 2)) + list(range(1, 18, 2)) + list(range(18, 27))
    w_off_p = w_off[perm]          # [27, 64, 3, 3] rows = dy(9), dx(9), logit(9)
    b_off_p = b_off[perm]
    # paired-tap weight packing: [128 partitions, n_mm * 32]
    woff_host = np.zeros((128, len(OFF_MMS) * 32), np.float16)
    for mi, (ks_mm, _ki, _c0) in enumerate(OFF_MMS):
        for j, k in enumerate(ks_mm):
            kyi, kxi = k // 3, k % 3
            woff_host[j * 64:(j + 1) * 64, mi * 32:mi * 32 + 27] = \
                w_off_p[:, :, kyi, kxi].T.astype(np.float16)
    b32 = np.zeros(32, np.float32)
    b32[:27] = b_off_p
    boff_host = np.tile(b32, 16).astype(np.float16).reshape(1, 512)
    wdr = w_dcn.reshape(COUT, CIN, KK)
    wy_host = np.zeros((KK, CIN, 64), np.float16)
    for k in range(KK):
        wy_host[k, :, :] = wdr[:, :, k].T.astype(np.float16)
    wy_host = np.ascontiguousarray(wy_host.transpose(1, 0, 2).reshape(CIN, KK * 64))
    ident_host = np.zeros((128, 132), np.float16)
    for p in range(128):
        ident_host[p, p + 2] = 1.0
    # fp8 ident blob (shipped as f32, cast in the load DMA):
    # S[k, m] = 1 iff m = k - a  (out[h] = tmp[h + a])
    def ident_a(a):
        I = np.zeros((128, 128), np.float32)
        for k in range(128):
            m = k - a
            if 0 <= m < 128:
                I[k, m] = 1.0
        return I
    blobs = [ident_a(a) for a in (-2, -1, 0, 1, 2)]
    for (a0, a1) in _PACKS:
        p = np.zeros((128, 2, 128), np.float32)
        p[:, 0, :] = ident_a(a0)
        p[:, 1, :] = ident_a(a1)
        blobs.append(p.reshape(128, 256))
    id8_host = np.concatenate(blobs, axis=1).astype(np.float32)
    return woff_host, boff_host, wy_host, ident_host, id8_host


def kernel(x, w_off, b_off, w_dcn):
    from concourse.bass_utils import run_bass_kernel_spmd

    if "nc" not in _NC_CACHE:
        _NC_CACHE["nc"] = _build_nc()
    nc = _NC_CACHE["nc"]

    woff_host, boff_host, wy_host, ident_host, id8_host = _prep_weights(
        np.asarray(w_off, np.float32), np.asarray(b_off, np.float32),
        np.asarray(w_dcn, np.float32))
    x = np.asarray(x, np.float32)
    x16 = x.astype(np.float16)
    in_maps = [{
        "x": np.ascontiguousarray(x16[b].reshape(CIN, HW)),
        "woff": woff_host, "boff": boff_host, "wy": wy_host,
        "ident": ident_host, "ident8": id8_host,
    } for b in range(B)]
    import os
    import time
    os.environ.setdefault("BASS_NEVER_TRACE", "1")
    res = None
    for attempt in range(3):
        try:
            res = run_bass_kernel_spmd(nc, in_maps, core_ids=list(range(B)))
            break
        except Exception:
            # transient NRT device errors clear on retry
            if attempt == 2:
                raise
            time.sleep(10)
    _NC_CACHE["last_results"] = res
    out = np.stack([res.results[b]["out"].reshape(COUT, H, W) for b in range(B)])
    out = out.astype(np.float32)
    _fixup_large_offsets(out, x, np.asarray(w_off, np.float32),
                         np.asarray(b_off, np.float32), np.asarray(w_dcn, np.float32))
    return out


def _fixup_large_offsets(out, x, w_off, b_off, w_dcn):
    """The on-device kernel uses a 3-tap tent decomposition of the bilinear
    interpolation, exact only for |offset| < 1. Offsets exceed 1 at ~1e-4 of
    sample points; recompute those output pixels exactly on host."""
    perm = list(range(0, 17, 2)) + list(range(1, 18, 2)) + list(range(18, 27))
    w_p = w_off[perm]
    b_p = b_off[perm]
    xpad = np.zeros((B, CIN, H + 2, W + 2), np.float32)
    xpad[:, :, 1:-1, 1:-1] = x
    off = np.zeros((B, 27, H, W), np.float32)
    for k in range(KK):
        kyi, kxi = k // 3, k % 3
        off += np.einsum("mc,bchw->bmhw", w_p[:, :, kyi, kxi],
                         xpad[:, :, kyi:kyi + H, kxi:kxi + W])
    off += b_p[None, :, None, None]
    dy, dx, lg = off[:, :9], off[:, 9:18], off[:, 18:27]
    bad = ((np.abs(dy) > 0.998) | (np.abs(dx) > 0.998)).any(axis=1)  # [B, H, W]
    if not bad.any():
        return
    wdr = w_dcn.reshape(COUT, CIN, KK)
    mask_all = 1.0 / (1.0 + np.exp(-lg))
    for b, h, w in zip(*np.nonzero(bad)):
        val = np.zeros((CIN, KK), np.float32)
        for k in range(KK):
            ki, kj = k // 3 - 1, k % 3 - 1
            py = h + ki + dy[b, k, h, w]
            px = w + kj + dx[b, k, h, w]
            y0, x0 = int(np.floor(py)), int(np.floor(px))
            wy1, wx1 = py - y0, px - x0
            acc = np.zeros(CIN, np.float32)
            for (yy, wyv) in ((y0, 1 - wy1), (y0 + 1, wy1)):
                for (xx, wxv) in ((x0, 1 - wx1), (x0 + 1, wx1)):
                    if 0 <= yy < H and 0 <= xx < W:
                        acc += np.float32(wyv * wxv) * x[b, :, yy, xx]
            val[:, k] = acc * mask_all[b, k, h, w]
        out[b, :, h, w] = np.einsum("ock,ck->o", wdr, val)
